# revision 13
# baseline (speedup 1.0000x reference)
"""Trainium2 Bass kernel for nn_DecoderLayer (self-attn + cross-attn + FFN).

Sharding: 8 cores = (batch b in 0..3) x (query-half in 0..1). Each core
computes 512 query tokens of one batch element end-to-end; K/V projections
over the full source sequence are duplicated across the two halves of a
batch element, so no collectives are needed.

v4 design (cost-model driven):
  - fp8e4m3 + DoubleRow (2 contraction k-tiles per instruction, 0.5
    cycles/row) for every D-contraction projection (q/k/v for both
    attentions). Weights are pre-scaled host-side (wkv*32, wq*4) so fp8
    operands sit in the normal range; the 2^-10 descale folds into the
    exp's scale and the softmax-denominator ones-column (=32).
  - fp8 scores (k,q evicted as fp8; softmax averaging absorbs the ~4%
    quantization noise). exp -> bf16 probabilities; multiplicative keep
    mask on DVE in the 2-byte 2x mode; bf16 ctx matmuls.
  - out-projections and the FFN stay bf16 (their noise lands directly on
    the output with no averaging, fp8 would blow the error budget).
  - stage-2's K/V projections are emitted INSIDE stage-1's attention
    window: the window is paced by the ACT engine's exps, so the PE would
    otherwise idle there. Engine streams are in-order, so chunks are
    interleaved between each head-pair's scores and ctx.
  - psum evictions spread across engines by phase: ACT evicts during
    projection phases (idle then), gpsimd during attention windows (ACT
    is busy with exps), DVE takes masks/normalize/residual/LN.
  - out-projection + LN + transpose run per-token-block so the tail
    pipelines; FFN weights prefetch during the preceding phase.

Self-contained: hardcodes all shapes; no sibling imports.
"""

import numpy as np
import ml_dtypes
from contextlib import ExitStack

import concourse.bass as bass
import concourse.tile as tile
from concourse import bacc, mybir
from concourse.bass_utils import run_bass_kernel_spmd
from concourse.masks import make_identity

P = 128
LN_EPS = 1e-5

F32 = mybir.dt.float32
BF16 = mybir.dt.bfloat16
FP8 = mybir.dt.float8e4

AF = mybir.ActivationFunctionType
ALU = mybir.AluOpType
DR = mybir.MatmulPerfMode.DoubleRow

EXP_SCALE = 1.0 / 1024.0  # descale the (32q).(32k) fp8 score accumulation


def build_decoder_nc(D=1024, S=1024, TP=512, H=16, FF=4096):
    dh = 64
    KC = D // P      # contraction chunks over D
    KC2 = KC // 2    # DoubleRow chunk pairs
    SB = S // P      # source blocks
    TB = TP // P     # query-token blocks
    NQ = TP
    FFC = FF // P
    HPV = 512 // dh  # heads per v-proj half
    assert NQ <= 512 and H * dh == D

    nc = bacc.Bacc("TRN2", target_bir_lowering=False, debug=False)

    def din(name, shape, dt):
        return nc.dram_tensor(name, shape, dt, kind="ExternalInput").ap()

    xqT_d = din("xqT", [D, TP], FP8)
    xq_d = din("xq", [TP, D], BF16)
    xfT_d = din("xfT", [D, S], FP8)
    encT_d = din("encT", [D, S], FP8)
    k1T_d = din("k1T", [S, TP], BF16)
    k2T_d = din("k2T", [S, TP], BF16)
    wq1_d = din("wq1", [D, D], FP8)        # pre-scaled 4x (incl dh**-0.5)
    wkv1_d = din("wkv1", [D, 2 * D], FP8)  # pre-scaled 32x
    wo1_d = din("wo1", [D, D], BF16)
    wq2_d = din("wq2", [D, D], FP8)
    wkv2_d = din("wkv2", [D, 2 * D], FP8)
    wo2_d = din("wo2", [D, D], BF16)
    win_d = din("w_in", [D, FF], BF16)
    wout_d = din("w_out", [FF, D], BF16)
    out_d = nc.dram_tensor("out", [TP, D], F32, kind="ExternalOutput").ap()

    with tile.TileContext(nc) as tc:
        with ExitStack() as ctx:
            # ---- bottom persistent pools (alive through FFN) ----
            consts = ctx.enter_context(tc.tile_pool(name="consts", bufs=1))
            p_stat = ctx.enter_context(tc.tile_pool(name="p_stat", bufs=10))
            p_res = ctx.enter_context(tc.tile_pool(name="p_res", bufs=3))
            p_resf = ctx.enter_context(tc.tile_pool(name="p_resf", bufs=2))
            p_xT = ctx.enter_context(tc.tile_pool(name="p_xT", bufs=1))
            p_ctxT = ctx.enter_context(tc.tile_pool(name="p_ctxT", bufs=1))
            p_wo = ctx.enter_context(tc.tile_pool(name="p_wo", bufs=1))
            pp_sc = ctx.enter_context(
                tc.tile_pool(name="pp_sc", bufs=2, space="PSUM"))
            pp_ctx = ctx.enter_context(
                tc.tile_pool(name="pp_ctx", bufs=3, space="PSUM"))
            pp_tr = ctx.enter_context(
                tc.tile_pool(name="pp_tr", bufs=1, space="PSUM"))

            ident = consts.tile([P, P], BF16)
            make_identity(nc, ident)
            eps_t = consts.tile([P, 1], F32)
            nc.vector.memset(eps_t, LN_EPS)

            def dr_chain(ps, lhs_t, l0, l1, rhs_t, r0, r1):
                """4 DoubleRow matmuls accumulating lhsT.T@rhs over D."""
                for kc2 in range(KC2):
                    k = 2 * kc2
                    nc.tensor.matmul(
                        ps, lhs_t[:, k:k + 2, l0:l1], rhs_t[:, k:k + 2, r0:r1],
                        start=(kc2 == 0), stop=(kc2 == KC2 - 1), perf_mode=DR)

            def ln_tb(res, xout, tb):
                st = p_stat.tile([P, 2, 6], F32, tag="lnst")
                for g in range(2):
                    nc.vector.bn_stats(st[:, g, :],
                                       res[:, tb, g * 512:(g + 1) * 512])
                mv = p_stat.tile([P, 2], F32, tag="lnmv")
                nc.vector.bn_aggr(mv, st)
                std = p_stat.tile([P, 1], F32, tag="lnstd")
                nc.scalar.activation(std, mv[:, 1:2], AF.Sqrt, bias=eps_t)
                rstd = p_stat.tile([P, 1], F32, tag="lnrstd")
                nc.vector.reciprocal(rstd, std)
                nc.vector.tensor_scalar(
                    out=xout[:, tb, :], in0=res[:, tb, :],
                    scalar1=mv[:, 0:1], scalar2=rstd,
                    op0=ALU.subtract, op1=ALU.mult)

            def tr_tb(src, dstT, tb, evict):
                for fc in range(KC):
                    ps = pp_tr.tile([P, P], BF16)
                    nc.tensor.transpose(
                        ps, src[:, tb, fc * P:(fc + 1) * P], ident)
                    dst = dstT[:, fc, tb * P:(tb + 1) * P]
                    if evict == "act":
                        nc.scalar.copy(dst, ps)
                    else:
                        nc.vector.tensor_copy(dst, ps)

            # ================= attention stack =================
            with ExitStack() as attn:
                p_kT = attn.enter_context(tc.tile_pool(name="p_kT", bufs=2))
                p_vt = attn.enter_context(tc.tile_pool(name="p_vt", bufs=2))
                p_qT = attn.enter_context(tc.tile_pool(name="p_qT", bufs=1))
                p_mT = attn.enter_context(tc.tile_pool(name="p_mT", bufs=1))
                p_kvs = attn.enter_context(tc.tile_pool(name="p_kvs", bufs=1))
                p_wkv = attn.enter_context(tc.tile_pool(name="p_wkv", bufs=1))
                p_wq = attn.enter_context(tc.tile_pool(name="p_wq", bufs=1))
                p_rsrc = attn.enter_context(
                    tc.tile_pool(name="p_rsrc", bufs=1))
                p_ets = attn.enter_context(tc.tile_pool(name="p_ets", bufs=16))

                class Stage:
                    pass

                def new_stage(kvT_dram, wkv_dram, mT_dram):
                    st = Stage()
                    st.kT = p_kT.tile([P, KC, S], FP8, tag="kT")
                    st.vt = p_vt.tile([P, SB, H, dh + 1], BF16, tag="vt")
                    st.qT = p_qT.tile([P, KC, NQ], FP8, tag="qT")
                    st.mT = p_mT.tile([P, SB, NQ], BF16, tag="mT")
                    st.kvs = p_kvs.tile([P, KC, S], FP8, tag="kvs")
                    st.wkv = p_wkv.tile([P, KC, 2 * D], FP8, tag="wkv")
                    st.ets = {}
                    nc.vector.memset(st.vt[:, :, :, dh:dh + 1], 32.0)
                    nc.sync.dma_start(
                        st.kvs, kvT_dram.rearrange("(c p) s -> p c s", p=P))
                    wr = wkv_dram.rearrange("(c p) m -> p c m", p=P)
                    nc.sync.dma_start(st.wkv[:, :, 0:D], wr[:, :, 0:D])
                    nc.sync.dma_start(st.wkv[:, :, D:2 * D], wr[:, :, D:2 * D])
                    nc.sync.dma_start(
                        st.mT, mT_dram.rearrange("(sb p) t -> p sb t", p=P))
                    return st

                def kT_chunk(st, of, evict):
                    kk = pp_sc.tile([P, 2, 512], F32, tag="ps", name="ps")
                    for sh in range(2):
                        dr_chain(kk[:, sh, :], st.wkv, of * P, (of + 1) * P,
                                 st.kvs, sh * 512, (sh + 1) * 512)
                    src = kk.rearrange("p a b -> p (a b)")
                    dst = st.kT[:, of, :]
                    if evict == "act":
                        nc.scalar.copy(dst, src)
                    else:
                        nc.vector.tensor_copy(dst, src)

                def v_chunk(st, vh, sbp, evict):
                    vv = pp_sc.tile([P, 2, 512], F32, tag="ps", name="ps")
                    for i in range(2):
                        sb = 2 * sbp + i
                        dr_chain(vv[:, i, :], st.kvs, sb * P, (sb + 1) * P,
                                 st.wkv, D + vh * 512, D + (vh + 1) * 512)
                    src = vv.rearrange("p a (h d) -> p a h d", d=dh)
                    dst = st.vt[:, 2 * sbp:2 * sbp + 2,
                                vh * HPV:(vh + 1) * HPV, 0:dh]
                    if evict == "act":
                        nc.scalar.copy(dst, src)
                    else:
                        nc.vector.tensor_copy(dst, src)

                def qT_pair(st, of, qsrc, evict):
                    qq = pp_sc.tile([P, 2, NQ], F32, tag="ps", name="ps")
                    for j in range(2):
                        dr_chain(qq[:, j, :], st.wq, (of + j) * P,
                                 (of + j + 1) * P, qsrc, 0, NQ)
                    dst = st.qT[:, of:of + 2, :]
                    if evict == "act":
                        nc.scalar.copy(dst, qq)
                    else:
                        nc.vector.tensor_copy(dst, qq)

                def sc_tile(st, h, sbp):
                    kc_h, ko = divmod(h * dh, P)
                    sc = pp_sc.tile([P, 2, NQ], F32, tag="ps", name="ps")
                    for i in range(2):
                        sb = 2 * sbp + i
                        nc.tensor.matmul(
                            sc[:, i, :],
                            st.kT[ko:ko + dh, kc_h, sb * P:(sb + 1) * P],
                            st.qT[ko:ko + dh, kc_h, :],
                            start=True, stop=True)
                    et = p_ets.tile([P, 2, NQ], BF16, name="et")
                    nc.scalar.activation(et, sc, AF.Exp, scale=EXP_SCALE)
                    eng = (nc.gpsimd if (h * 4 + sbp) % 3 == 0
                           else nc.vector)
                    eng.tensor_mul(et, et, st.mT[:, 2 * sbp:2 * sbp + 2, :])
                    st.ets[(h, sbp)] = et

                def ctx_chain(st, ctxt, h, tb):
                    psc = pp_ctx.tile([P, dh + 1], F32, name="psc")
                    n = 0
                    for sbp in range(SB // 2):
                        for i in range(2):
                            nc.tensor.matmul(
                                psc,
                                st.ets[(h, sbp)][:, i, tb * P:(tb + 1) * P],
                                st.vt[:, 2 * sbp + i, h, :],
                                start=(n == 0), stop=(n == SB - 1))
                            n += 1
                    nc.vector.tensor_scalar(
                        out=ctxt[:, tb, h * dh:(h + 1) * dh],
                        in0=psc[:, 0:dh], scalar1=psc[:, dh:dh + 1],
                        scalar2=None, op0=ALU.divide)

                def window(st, ctxt, extra_per_hp):
                    """Attention window: 8 slots per head-pair; each slot
                    emits [prev ctx chain][extra work][scores tile] so the
                    PE fills the exp-paced psum-ring waits."""
                    prev = None
                    for hpi, hp in enumerate(range(0, H, 2)):
                        sc_list = [(h, sbp) for h in (hp, hp + 1)
                                   for sbp in range(SB // 2)]
                        cx_list = ([] if prev is None else
                                   [(h, tb) for h in (prev, prev + 1)
                                    for tb in range(TB)])
                        extra = list(extra_per_hp[hpi])
                        for k in range(8):
                            if k < len(cx_list):
                                ctx_chain(st, ctxt, *cx_list[k])
                            if k >= 3 and extra:
                                extra.pop(0)()
                            sc_tile(st, *sc_list[k])
                        for em in extra:
                            em()
                        prev = hp
                    for h in (prev, prev + 1):
                        for tb in range(TB):
                            ctx_chain(st, ctxt, h, tb)

                def out_stage(ctxT, wot, rsrc, res, xo, xT, tr_evict):
                    for tb in range(TB):
                        po = pp_sc.tile([P, 2, 512], F32, tag="ps", name="ps")
                        for fc in range(KC):
                            for oh in range(2):
                                nc.tensor.matmul(
                                    po[:, oh, :],
                                    ctxT[:, fc, tb * P:(tb + 1) * P],
                                    wot[:, fc, oh * 512:(oh + 1) * 512],
                                    start=(fc == 0), stop=(fc == KC - 1))
                        nc.vector.tensor_add(
                            res[:, tb, :], po.rearrange("p a b -> p (a b)"),
                            rsrc[:, tb, :])
                        ln_tb(res, xo, tb)
                        if tb >= 1:
                            tr_tb(xo, xT, tb - 1, tr_evict)
                    tr_tb(xo, xT, TB - 1, tr_evict)

                # ---------------- DMA prefetch + stage tiles ----------
                s1 = new_stage(xfT_d, wkv1_d, k1T_d)
                s1.wq = p_wq.tile([P, KC, D], FP8, tag="wq")
                nc.sync.dma_start(
                    s1.wq, wq1_d.rearrange("(c p) m -> p c m", p=P))


                s2 = new_stage(encT_d, wkv2_d, k2T_d)
                s2.wq = p_wq.tile([P, KC, D], FP8, tag="wq")
                nc.sync.dma_start(
                    s2.wq, wq2_d.rearrange("(c p) m -> p c m", p=P))

                wo1 = p_wo.tile([P, KC, D], BF16, tag="wo")
                nc.sync.dma_start(
                    wo1, wo1_d.rearrange("(c p) m -> p c m", p=P))
                rsrc1 = p_rsrc.tile([P, TB, D], BF16, tag="rsrc")
                nc.sync.dma_start(
                    rsrc1, xq_d.rearrange("(tb p) d -> p tb d", p=P))

                # ---------------- s1 projections ----------------
                with tc.tile_pool(name="p_qsrc", bufs=1) as p_qsrc:
                    qsrc = p_qsrc.tile([P, KC, NQ], FP8, tag="qsrc")
                    nc.sync.dma_start(
                        qsrc, xqT_d.rearrange("(c p) t -> p c t", p=P))
                    for of in range(KC):
                        kT_chunk(s1, of, evict="act")
                    for vh in range(2):
                        for sbp in range(SB // 2):
                            v_chunk(s1, vh, sbp, evict="dve")
                    for of in range(0, KC, 2):
                        qT_pair(s1, of, qsrc, evict="dve")

                # ---------------- window 1: s1 attention + s2 kv ------
                ctxt1 = p_res.tile([P, TB, D], BF16, tag="res")
                kv2_work = [lambda of=of: kT_chunk(s2, of, evict="dve")
                            for of in range(KC)]
                kv2_work += [lambda vh=vh, sbp=sbp:
                             v_chunk(s2, vh, sbp, evict="dve")
                             for vh in range(2) for sbp in range(SB // 2)]
                kv2_counts = [0, 2, 2, 2, 2, 2, 3, 3]
                extra1 = []
                wi = 0
                for c in kv2_counts:
                    extra1.append(kv2_work[wi:wi + c])
                    wi += c
                window(s1, ctxt1, extra1)

                # ---------------- s1 out-projection ----------------
                ctxT1 = p_ctxT.tile([P, KC, TP], BF16, tag="ctxT")
                for tb in range(TB):
                    tr_tb(ctxt1, ctxT1, tb, "dve")
                res1 = p_res.tile([P, TB, D], BF16, tag="res")
                x1 = p_res.tile([P, TB, D], BF16, tag="res")
                x1T = p_xT.tile([P, KC, TP], FP8, tag="xT")
                out_stage(ctxT1, wo1, rsrc1, res1, x1, x1T, "act")

                wo2 = p_wo.tile([P, KC, D], BF16, tag="wo")
                nc.sync.dma_start(
                    wo2, wo2_d.rearrange("(c p) m -> p c m", p=P))

                # ---------------- window 2: s2 attention ----------------
                ctxt2 = p_res.tile([P, TB, D], BF16, tag="res")
                qT_pair(s2, 0, x1T, evict="dve")
                qT_pair(s2, 2, x1T, evict="dve")
                extra2 = [[] for _ in range(8)]
                extra2[1] = [lambda: qT_pair(s2, 4, x1T, evict="dve")]
                extra2[3] = [lambda: qT_pair(s2, 6, x1T, evict="dve")]
                window(s2, ctxt2, extra2)

                ctxT2 = p_ctxT.tile([P, KC, TP], BF16, tag="ctxT")
                for tb in range(TB):
                    tr_tb(ctxt2, ctxT2, tb, "dve")
            # attention stack closed: kv/weights/ets SBUF freed

            # ---------------- FFN weight prefetch ----------------
            p_hT = ctx.enter_context(tc.tile_pool(name="p_hT", bufs=1))
            hT = p_hT.tile([P, FFC, NQ], BF16)
            wir = win_d.rearrange("(c p) m -> p c m", p=P)
            wor = wout_d.rearrange("(c p) m -> p c m", p=P)
            with tc.tile_pool(name="p_wit", bufs=2) as p_wit, \
                    tc.tile_pool(name="p_wot", bufs=2) as p_wot:
                NWQ = 4  # w_in quarter chunks, ring of 2
                QW = FF // NWQ
                wits = []
                for q in range(2):
                    w = p_wit.tile([P, KC, QW], BF16, tag="wit")
                    nc.sync.dma_start(w, wir[:, :, q * QW:(q + 1) * QW])
                    wits.append(w)
                wots = []
                for oh in range(2):
                    w = p_wot.tile([P, FFC, 512], BF16, tag="wot")
                    nc.sync.dma_start(w, wor[:, :, oh * 512:(oh + 1) * 512])
                    wots.append(w)

                # ---------------- s2 out-projection ----------------
                res2 = p_res.tile([P, TB, D], BF16, tag="res")
                x2 = p_res.tile([P, TB, D], BF16, tag="res")
                x2T = p_xT.tile([P, KC, TP], BF16, tag="xT")
                out_stage(ctxT2, wo2, x1, res2, x2, x2T, "act")

                # ---------------- FFN hidden ----------------
                FPQ = QW // (2 * P)  # ffc-pairs per quarter
                for fp in range(FFC // 2):
                    q = fp // FPQ
                    if q >= 2 and fp % FPQ == 0:
                        w = p_wit.tile([P, KC, QW], BF16, tag="wit")
                        nc.sync.dma_start(
                            w, wir[:, :, q * QW:(q + 1) * QW])
                        wits.append(w)
                    hh = pp_sc.tile([P, 2, NQ], F32, tag="ps", name="ps")
                    wt = wits[q]
                    base = q * QW
                    for j in range(2):
                        c0 = (2 * fp + j) * P - base
                        for kc in range(KC):
                            nc.tensor.matmul(
                                hh[:, j, :], wt[:, kc, c0:c0 + P],
                                x2T[:, kc, :],
                                start=(kc == 0), stop=(kc == KC - 1))
                    nc.scalar.activation(hT[:, 2 * fp:2 * fp + 2, :], hh,
                                         AF.Relu)
                res3 = p_res.tile([P, TB, D], BF16, tag="res")
                outr = out_d.rearrange("(tb p) d -> p tb d", p=P)
                for oh in range(2):
                    for tbp in range(TB // 2):
                        po = pp_sc.tile([P, 2, 512], F32, tag="ps", name="ps")
                        for ffc in range(FFC):
                            for i in range(2):
                                tb = 2 * tbp + i
                                nc.tensor.matmul(
                                    po[:, i, :],
                                    hT[:, ffc, tb * P:(tb + 1) * P],
                                    wots[oh][:, ffc, :],
                                    start=(ffc == 0), stop=(ffc == FFC - 1))
                        for i in range(2):
                            tb = 2 * tbp + i
                            nc.vector.tensor_add(
                                res3[:, tb, oh * 512:(oh + 1) * 512],
                                po[:, i, :],
                                x2[:, tb, oh * 512:(oh + 1) * 512])
                            if oh == 1:
                                xot = p_resf.tile([P, 1, D], F32, tag="resf",
                                                  name="xot")
                                # per-tb LN into a [P,1,D] staging tile
                                stv = p_stat.tile([P, 2, 6], F32, tag="lnst")
                                for g in range(2):
                                    nc.vector.bn_stats(
                                        stv[:, g, :],
                                        res3[:, tb, g * 512:(g + 1) * 512])
                                mv = p_stat.tile([P, 2], F32, tag="lnmv")
                                nc.vector.bn_aggr(mv, stv)
                                std = p_stat.tile([P, 1], F32, tag="lnstd")
                                nc.scalar.activation(std, mv[:, 1:2],
                                                     AF.Sqrt, bias=eps_t)
                                rstd = p_stat.tile([P, 1], F32, tag="lnrstd")
                                nc.vector.reciprocal(rstd, std)
                                nc.vector.tensor_scalar(
                                    out=xot[:, 0, :], in0=res3[:, tb, :],
                                    scalar1=mv[:, 0:1], scalar2=rstd,
                                    op0=ALU.subtract, op1=ALU.mult)
                                nc.sync.dma_start(outr[:, tb, :],
                                                  xot[:, 0, :])

    nc.compile()
    return nc


# ---------------------------------------------------------------------------
# host side
# ---------------------------------------------------------------------------

_NC_CACHE = {}

MM_KEY = ("v4",)


def _get_nc(key=MM_KEY):
    if key not in _NC_CACHE:
        _NC_CACHE[key] = build_decoder_nc()
    return _NC_CACHE[key]


def _numpy_reference(x, enc_out, src_mask, tgt_mask, wq1, bq1, wkv1, bkv1,
                     wo1, bo1, wq2, bq2, wkv2, bkv2, wo2, bo2, w_in, b_in,
                     w_out, b_out, g0, be0, g1, be1, g2, be2):
    """Pure-numpy fallback (exact reference semantics)."""
    H, D = 16, 1024

    def ln(x, g, b):
        m = x.mean(-1, keepdims=True)
        v = ((x - m) ** 2).mean(-1, keepdims=True)
        return (x - m) / np.sqrt(v + LN_EPS) * g + b

    def attn(q_in, mem, mask, wq, bq, wkv, bkv, wo, bo):
        B, T, _ = q_in.shape
        S = mem.shape[1]
        dhl = D // H
        q = (q_in @ wq + bq).reshape(B, T, H, dhl) * (dhl ** -0.5)
        k, v = np.split(mem @ wkv + bkv, 2, axis=-1)
        k = k.reshape(B, S, H, dhl)
        v = v.reshape(B, S, H, dhl)
        sc = np.einsum('bthd,bshd->bhts', q, k)
        sc = np.where(mask[:, None, :, :], -1e20, sc)
        sc = sc - sc.max(-1, keepdims=True)
        w = np.exp(sc)
        w = w / w.sum(-1, keepdims=True)
        ctx = np.einsum('bhts,bshd->bthd', w, v).reshape(B, T, D)
        return ctx @ wo + bo

    y = attn(x, x, tgt_mask, wq1, bq1, wkv1, bkv1, wo1, bo1)
    x1 = ln(x + y, g0, be0)
    y = attn(x1, enc_out, src_mask, wq2, bq2, wkv2, bkv2, wo2, bo2)
    x2 = ln(x1 + y, g1, be1)
    y = np.maximum(x2 @ w_in + b_in, 0.0) @ w_out + b_out
    return ln(x2 + y, g2, be2)


def kernel(x, enc_out, src_mask, tgt_mask, wq1, bq1, wkv1, bkv1, wo1, bo1,
           wq2, bq2, wkv2, bkv2, wo2, bo2, w_in, b_in, w_out, b_out,
           g0, be0, g1, be1, g2, be2, _trace=False):
    x = np.asarray(x)
    args = dict(x=x, enc_out=np.asarray(enc_out),
                src_mask=np.asarray(src_mask), tgt_mask=np.asarray(tgt_mask),
                wq1=np.asarray(wq1), bq1=np.asarray(bq1),
                wkv1=np.asarray(wkv1), bkv1=np.asarray(bkv1),
                wo1=np.asarray(wo1), bo1=np.asarray(bo1),
                wq2=np.asarray(wq2), bq2=np.asarray(bq2),
                wkv2=np.asarray(wkv2), bkv2=np.asarray(bkv2),
                wo2=np.asarray(wo2), bo2=np.asarray(bo2),
                w_in=np.asarray(w_in), b_in=np.asarray(b_in),
                w_out=np.asarray(w_out), b_out=np.asarray(b_out),
                g0=np.asarray(g0), be0=np.asarray(be0),
                g1=np.asarray(g1), be1=np.asarray(be1),
                g2=np.asarray(g2), be2=np.asarray(be2))

    # the hardware kernel folds out zero biases / unit gains (true for this
    # problem's setup_inputs); anything else falls back to exact numpy.
    zeros = [args[k] for k in ("bq1", "bkv1", "bo1", "bq2", "bkv2", "bo2",
                               "b_in", "b_out", "be0", "be1", "be2")]
    ones = [args["g0"], args["g1"], args["g2"]]
    if any(np.any(z != 0) for z in zeros) or any(np.any(g != 1) for g in ones):
        res = _numpy_reference(**args)
        return res.astype(np.float32), x

    B, T, D = x.shape
    TP = T // 2
    bf = ml_dtypes.bfloat16
    f8 = ml_dtypes.float8_e4m3

    def cbf(a):
        return np.ascontiguousarray(a.astype(bf))

    def c8(a):
        return np.ascontiguousarray(a.astype(f8))

    wq1b = c8(args["wq1"] * np.float32(4.0))
    wq2b = c8(args["wq2"] * np.float32(4.0))
    wkv1b = c8(args["wkv1"] * np.float32(32.0))
    wkv2b = c8(args["wkv2"] * np.float32(32.0))
    wo1b = cbf(args["wo1"])
    wo2b = cbf(args["wo2"])
    w_inb = cbf(args["w_in"])
    w_outb = cbf(args["w_out"])

    in_maps = []
    for core in range(8):
        b, half = divmod(core, 2)
        t0 = half * TP
        xb = args["x"][b]
        xs = xb[t0:t0 + TP]
        in_maps.append({
            "xqT": c8(xs.T),
            "xq": cbf(xs),
            "xfT": c8(xb.T),
            "encT": c8(args["enc_out"][b].T),
            "k1T": cbf((~args["tgt_mask"][b, t0:t0 + TP]).T
                       .astype(np.float32)),
            "k2T": cbf((~args["src_mask"][b, t0:t0 + TP]).T
                       .astype(np.float32)),
            "wq1": wq1b,
            "wkv1": wkv1b,
            "wo1": wo1b,
            "wq2": wq2b,
            "wkv2": wkv2b,
            "wo2": wo2b,
            "w_in": w_inb,
            "w_out": w_outb,
        })

    nc = _get_nc(MM_KEY)
    res = run_bass_kernel_spmd(nc, in_maps, core_ids=list(range(8)),
                               trace=_trace)
    outp = np.empty((B, T, D), np.float32)
    for core in range(8):
        b, half = divmod(core, 2)
        outp[b, half * TP:(half + 1) * TP] = res.results[core]["out"]
    if _trace:
        kernel.last_results = res
    return outp, x


# revision 14
# speedup vs baseline: 1.0024x; 1.0024x over previous
"""Trainium2 Bass kernel for nn_DecoderLayer (self-attn + cross-attn + FFN).

Sharding: 8 cores = (batch b in 0..3) x (query-half in 0..1). Each core
computes 512 query tokens of one batch element end-to-end; K/V projections
over the full source sequence are duplicated across the two halves of a
batch element, so no collectives are needed.

v4 design (cost-model driven):
  - fp8e4m3 + DoubleRow (2 contraction k-tiles per instruction, 0.5
    cycles/row) for every D-contraction projection (q/k/v for both
    attentions). Weights are pre-scaled host-side (wkv*32, wq*4) so fp8
    operands sit in the normal range; the 2^-10 descale folds into the
    exp's scale and the softmax-denominator ones-column (=32).
  - fp8 scores (k,q evicted as fp8; softmax averaging absorbs the ~4%
    quantization noise). exp -> bf16 probabilities; multiplicative keep
    mask on DVE in the 2-byte 2x mode; bf16 ctx matmuls.
  - out-projections and the FFN stay bf16 (their noise lands directly on
    the output with no averaging, fp8 would blow the error budget).
  - stage-2's K/V projections are emitted INSIDE stage-1's attention
    window: the window is paced by the ACT engine's exps, so the PE would
    otherwise idle there. Engine streams are in-order, so chunks are
    interleaved between each head-pair's scores and ctx.
  - psum evictions spread across engines by phase: ACT evicts during
    projection phases (idle then), gpsimd during attention windows (ACT
    is busy with exps), DVE takes masks/normalize/residual/LN.
  - out-projection + LN + transpose run per-token-block so the tail
    pipelines; FFN weights prefetch during the preceding phase.

Self-contained: hardcodes all shapes; no sibling imports.
"""

import numpy as np
import ml_dtypes
from contextlib import ExitStack

import concourse.bass as bass
import concourse.tile as tile
from concourse import bacc, mybir
from concourse.bass_utils import run_bass_kernel_spmd
from concourse.masks import make_identity

P = 128
LN_EPS = 1e-5

F32 = mybir.dt.float32
BF16 = mybir.dt.bfloat16
FP8 = mybir.dt.float8e4

AF = mybir.ActivationFunctionType
ALU = mybir.AluOpType
DR = mybir.MatmulPerfMode.DoubleRow

EXP_SCALE = 1.0 / 1024.0  # descale the (32q).(32k) fp8 score accumulation


def build_decoder_nc(D=1024, S=1024, TP=512, H=16, FF=4096):
    dh = 64
    KC = D // P      # contraction chunks over D
    KC2 = KC // 2    # DoubleRow chunk pairs
    SB = S // P      # source blocks
    TB = TP // P     # query-token blocks
    NQ = TP
    FFC = FF // P
    HPV = 512 // dh  # heads per v-proj half
    assert NQ <= 512 and H * dh == D

    nc = bacc.Bacc("TRN2", target_bir_lowering=False, debug=False)

    def din(name, shape, dt):
        return nc.dram_tensor(name, shape, dt, kind="ExternalInput").ap()

    xqT_d = din("xqT", [D, TP], FP8)
    xq_d = din("xq", [TP, D], BF16)
    xfT_d = din("xfT", [D, S], FP8)
    encT_d = din("encT", [D, S], FP8)
    k1T_d = din("k1T", [S, TP], BF16)
    k2T_d = din("k2T", [S, TP], BF16)
    wq1_d = din("wq1", [D, D], FP8)        # pre-scaled 4x (incl dh**-0.5)
    wkv1_d = din("wkv1", [D, 2 * D], FP8)  # pre-scaled 32x
    wo1_d = din("wo1", [D, D], BF16)
    wq2_d = din("wq2", [D, D], FP8)
    wkv2_d = din("wkv2", [D, 2 * D], FP8)
    wo2_d = din("wo2", [D, D], BF16)
    win_d = din("w_in", [D, FF], BF16)
    wout_d = din("w_out", [FF, D], BF16)
    out_d = nc.dram_tensor("out", [TP, D], F32, kind="ExternalOutput").ap()

    with tile.TileContext(nc) as tc:
        with ExitStack() as ctx:
            # ---- bottom persistent pools (alive through FFN) ----
            consts = ctx.enter_context(tc.tile_pool(name="consts", bufs=1))
            p_stat = ctx.enter_context(tc.tile_pool(name="p_stat", bufs=10))
            p_res = ctx.enter_context(tc.tile_pool(name="p_res", bufs=3))
            p_resf = ctx.enter_context(tc.tile_pool(name="p_resf", bufs=2))
            p_xT = ctx.enter_context(tc.tile_pool(name="p_xT", bufs=1))
            p_ctxT = ctx.enter_context(tc.tile_pool(name="p_ctxT", bufs=1))
            p_wo = ctx.enter_context(tc.tile_pool(name="p_wo", bufs=1))
            pp_sc = ctx.enter_context(
                tc.tile_pool(name="pp_sc", bufs=2, space="PSUM"))
            pp_ctx = ctx.enter_context(
                tc.tile_pool(name="pp_ctx", bufs=3, space="PSUM"))
            pp_tr = ctx.enter_context(
                tc.tile_pool(name="pp_tr", bufs=1, space="PSUM"))

            ident = consts.tile([P, P], BF16)
            make_identity(nc, ident)
            eps_t = consts.tile([P, 1], F32)
            nc.vector.memset(eps_t, LN_EPS)

            def dr_chain(ps, lhs_t, l0, l1, rhs_t, r0, r1):
                """4 DoubleRow matmuls accumulating lhsT.T@rhs over D."""
                for kc2 in range(KC2):
                    k = 2 * kc2
                    nc.tensor.matmul(
                        ps, lhs_t[:, k:k + 2, l0:l1], rhs_t[:, k:k + 2, r0:r1],
                        start=(kc2 == 0), stop=(kc2 == KC2 - 1), perf_mode=DR)

            def ln_tb(res, xout, tb):
                st = p_stat.tile([P, 2, 6], F32, tag="lnst")
                for g in range(2):
                    nc.vector.bn_stats(st[:, g, :],
                                       res[:, tb, g * 512:(g + 1) * 512])
                mv = p_stat.tile([P, 2], F32, tag="lnmv")
                nc.vector.bn_aggr(mv, st)
                std = p_stat.tile([P, 1], F32, tag="lnstd")
                nc.scalar.activation(std, mv[:, 1:2], AF.Sqrt, bias=eps_t)
                rstd = p_stat.tile([P, 1], F32, tag="lnrstd")
                nc.vector.reciprocal(rstd, std)
                nc.vector.tensor_scalar(
                    out=xout[:, tb, :], in0=res[:, tb, :],
                    scalar1=mv[:, 0:1], scalar2=rstd,
                    op0=ALU.subtract, op1=ALU.mult)

            def tr_tb(src, dstT, tb, evict):
                for fc in range(KC):
                    ps = pp_tr.tile([P, P], BF16)
                    nc.tensor.transpose(
                        ps, src[:, tb, fc * P:(fc + 1) * P], ident)
                    dst = dstT[:, fc, tb * P:(tb + 1) * P]
                    if evict == "act":
                        nc.scalar.copy(dst, ps)
                    else:
                        nc.vector.tensor_copy(dst, ps)

            # ================= attention stack =================
            with ExitStack() as attn:
                p_kT = attn.enter_context(tc.tile_pool(name="p_kT", bufs=2))
                p_vt = attn.enter_context(tc.tile_pool(name="p_vt", bufs=2))
                p_qT = attn.enter_context(tc.tile_pool(name="p_qT", bufs=1))
                p_mT = attn.enter_context(tc.tile_pool(name="p_mT", bufs=1))
                p_kvs = attn.enter_context(tc.tile_pool(name="p_kvs", bufs=1))
                p_wkv = attn.enter_context(tc.tile_pool(name="p_wkv", bufs=1))
                p_wq = attn.enter_context(tc.tile_pool(name="p_wq", bufs=1))
                p_rsrc = attn.enter_context(
                    tc.tile_pool(name="p_rsrc", bufs=1))
                p_ets = attn.enter_context(tc.tile_pool(name="p_ets", bufs=16))

                class Stage:
                    pass

                def new_stage(kvT_dram, wkv_dram, mT_dram):
                    st = Stage()
                    st.kT = p_kT.tile([P, KC, S], FP8, tag="kT")
                    st.vt = p_vt.tile([P, SB, H, dh + 1], BF16, tag="vt")
                    st.qT = p_qT.tile([P, KC, NQ], FP8, tag="qT")
                    st.mT = p_mT.tile([P, SB, NQ], BF16, tag="mT")
                    st.kvs = p_kvs.tile([P, KC, S], FP8, tag="kvs")
                    st.wkv = p_wkv.tile([P, KC, 2 * D], FP8, tag="wkv")
                    st.ets = {}
                    nc.vector.memset(st.vt[:, :, :, dh:dh + 1], 32.0)
                    nc.sync.dma_start(
                        st.kvs, kvT_dram.rearrange("(c p) s -> p c s", p=P))
                    wr = wkv_dram.rearrange("(c p) m -> p c m", p=P)
                    nc.sync.dma_start(st.wkv[:, :, 0:D], wr[:, :, 0:D])
                    nc.sync.dma_start(st.wkv[:, :, D:2 * D], wr[:, :, D:2 * D])
                    nc.sync.dma_start(
                        st.mT, mT_dram.rearrange("(sb p) t -> p sb t", p=P))
                    return st

                def kT_chunk(st, of, evict):
                    kk = pp_sc.tile([P, 2, 512], F32, tag="ps", name="ps")
                    for sh in range(2):
                        dr_chain(kk[:, sh, :], st.wkv, of * P, (of + 1) * P,
                                 st.kvs, sh * 512, (sh + 1) * 512)
                    src = kk.rearrange("p a b -> p (a b)")
                    dst = st.kT[:, of, :]
                    if evict == "act":
                        nc.scalar.copy(dst, src)
                    else:
                        nc.vector.tensor_copy(dst, src)

                def v_chunk(st, vh, sbp, evict):
                    vv = pp_sc.tile([P, 2, 512], F32, tag="ps", name="ps")
                    for i in range(2):
                        sb = 2 * sbp + i
                        dr_chain(vv[:, i, :], st.kvs, sb * P, (sb + 1) * P,
                                 st.wkv, D + vh * 512, D + (vh + 1) * 512)
                    src = vv.rearrange("p a (h d) -> p a h d", d=dh)
                    dst = st.vt[:, 2 * sbp:2 * sbp + 2,
                                vh * HPV:(vh + 1) * HPV, 0:dh]
                    if evict == "act":
                        nc.scalar.copy(dst, src)
                    else:
                        nc.vector.tensor_copy(dst, src)

                def qT_pair(st, of, qsrc, evict):
                    qq = pp_sc.tile([P, 2, NQ], F32, tag="ps", name="ps")
                    for j in range(2):
                        dr_chain(qq[:, j, :], st.wq, (of + j) * P,
                                 (of + j + 1) * P, qsrc, 0, NQ)
                    dst = st.qT[:, of:of + 2, :]
                    if evict == "act":
                        nc.scalar.copy(dst, qq)
                    else:
                        nc.vector.tensor_copy(dst, qq)

                def sc_tile(st, h, sbp):
                    kc_h, ko = divmod(h * dh, P)
                    sc = pp_sc.tile([P, 2, NQ], F32, tag="ps", name="ps")
                    for i in range(2):
                        sb = 2 * sbp + i
                        nc.tensor.matmul(
                            sc[:, i, :],
                            st.kT[ko:ko + dh, kc_h, sb * P:(sb + 1) * P],
                            st.qT[ko:ko + dh, kc_h, :],
                            start=True, stop=True)
                    et = p_ets.tile([P, 2, NQ], BF16, name="et")
                    nc.scalar.activation(et, sc, AF.Exp, scale=EXP_SCALE)
                    eng = (nc.gpsimd if (h * 4 + sbp) % 3 == 0
                           else nc.vector)
                    eng.tensor_mul(et, et, st.mT[:, 2 * sbp:2 * sbp + 2, :])
                    st.ets[(h, sbp)] = et

                def ctx_chain(st, ctxt, h, tb):
                    psc = pp_ctx.tile([P, dh + 1], F32, name="psc")
                    n = 0
                    for sbp in range(SB // 2):
                        for i in range(2):
                            nc.tensor.matmul(
                                psc,
                                st.ets[(h, sbp)][:, i, tb * P:(tb + 1) * P],
                                st.vt[:, 2 * sbp + i, h, :],
                                start=(n == 0), stop=(n == SB - 1))
                            n += 1
                    rec = p_stat.tile([P, 1], F32, tag="rec", name="rec")
                    nc.vector.reciprocal(rec, psc[:, dh:dh + 1])
                    nc.vector.tensor_scalar_mul(
                        ctxt[:, tb, h * dh:(h + 1) * dh],
                        in0=psc[:, 0:dh], scalar1=rec)

                def window(st, ctxt, extra_per_hp):
                    """Attention window: 8 slots per head-pair; each slot
                    emits [prev ctx chain][extra work][scores tile] so the
                    PE fills the exp-paced psum-ring waits."""
                    prev = None
                    for hpi, hp in enumerate(range(0, H, 2)):
                        sc_list = [(h, sbp) for h in (hp, hp + 1)
                                   for sbp in range(SB // 2)]
                        cx_list = ([] if prev is None else
                                   [(h, tb) for h in (prev, prev + 1)
                                    for tb in range(TB)])
                        extra = list(extra_per_hp[hpi])
                        for k in range(8):
                            if k < len(cx_list):
                                ctx_chain(st, ctxt, *cx_list[k])
                            if k >= 3 and extra:
                                extra.pop(0)()
                            sc_tile(st, *sc_list[k])
                        for em in extra:
                            em()
                        prev = hp
                    for h in (prev, prev + 1):
                        for tb in range(TB):
                            ctx_chain(st, ctxt, h, tb)

                def out_stage(ctxT, wot, rsrc, res, xo, xT, tr_evict):
                    for tb in range(TB):
                        po = pp_sc.tile([P, 2, 512], F32, tag="ps", name="ps")
                        for fc in range(KC):
                            for oh in range(2):
                                nc.tensor.matmul(
                                    po[:, oh, :],
                                    ctxT[:, fc, tb * P:(tb + 1) * P],
                                    wot[:, fc, oh * 512:(oh + 1) * 512],
                                    start=(fc == 0), stop=(fc == KC - 1))
                        nc.vector.tensor_add(
                            res[:, tb, :], po.rearrange("p a b -> p (a b)"),
                            rsrc[:, tb, :])
                        ln_tb(res, xo, tb)
                        if tb >= 1:
                            tr_tb(xo, xT, tb - 1, tr_evict)
                    tr_tb(xo, xT, TB - 1, tr_evict)

                # ---------------- DMA prefetch + stage tiles ----------
                s1 = new_stage(xfT_d, wkv1_d, k1T_d)
                s1.wq = p_wq.tile([P, KC, D], FP8, tag="wq")
                nc.sync.dma_start(
                    s1.wq, wq1_d.rearrange("(c p) m -> p c m", p=P))


                s2 = new_stage(encT_d, wkv2_d, k2T_d)
                s2.wq = p_wq.tile([P, KC, D], FP8, tag="wq")
                nc.sync.dma_start(
                    s2.wq, wq2_d.rearrange("(c p) m -> p c m", p=P))

                wo1 = p_wo.tile([P, KC, D], BF16, tag="wo")
                nc.sync.dma_start(
                    wo1, wo1_d.rearrange("(c p) m -> p c m", p=P))
                rsrc1 = p_rsrc.tile([P, TB, D], BF16, tag="rsrc")
                nc.sync.dma_start(
                    rsrc1, xq_d.rearrange("(tb p) d -> p tb d", p=P))

                # ---------------- s1 projections ----------------
                with tc.tile_pool(name="p_qsrc", bufs=1) as p_qsrc:
                    qsrc = p_qsrc.tile([P, KC, NQ], FP8, tag="qsrc")
                    nc.sync.dma_start(
                        qsrc, xqT_d.rearrange("(c p) t -> p c t", p=P))
                    for of in range(KC):
                        kT_chunk(s1, of, evict="act")
                    for vh in range(2):
                        for sbp in range(SB // 2):
                            v_chunk(s1, vh, sbp, evict="dve")
                    for of in range(0, KC, 2):
                        qT_pair(s1, of, qsrc, evict="dve")

                # ---------------- window 1: s1 attention + s2 kv ------
                ctxt1 = p_res.tile([P, TB, D], BF16, tag="res")
                kv2_work = [lambda of=of: kT_chunk(s2, of, evict="dve")
                            for of in range(KC)]
                kv2_work += [lambda vh=vh, sbp=sbp:
                             v_chunk(s2, vh, sbp, evict="dve")
                             for vh in range(2) for sbp in range(SB // 2)]
                kv2_counts = [0, 2, 2, 2, 2, 2, 3, 3]
                extra1 = []
                wi = 0
                for c in kv2_counts:
                    extra1.append(kv2_work[wi:wi + c])
                    wi += c
                window(s1, ctxt1, extra1)

                # ---------------- s1 out-projection ----------------
                ctxT1 = p_ctxT.tile([P, KC, TP], BF16, tag="ctxT")
                for tb in range(TB):
                    tr_tb(ctxt1, ctxT1, tb, "dve")
                res1 = p_res.tile([P, TB, D], BF16, tag="res")
                x1 = p_res.tile([P, TB, D], BF16, tag="res")
                x1T = p_xT.tile([P, KC, TP], FP8, tag="xT")
                out_stage(ctxT1, wo1, rsrc1, res1, x1, x1T, "act")

                wo2 = p_wo.tile([P, KC, D], BF16, tag="wo")
                nc.sync.dma_start(
                    wo2, wo2_d.rearrange("(c p) m -> p c m", p=P))

                # ---------------- window 2: s2 attention ----------------
                ctxt2 = p_res.tile([P, TB, D], BF16, tag="res")
                qT_pair(s2, 0, x1T, evict="dve")
                qT_pair(s2, 2, x1T, evict="dve")
                extra2 = [[] for _ in range(8)]
                extra2[1] = [lambda: qT_pair(s2, 4, x1T, evict="dve")]
                extra2[3] = [lambda: qT_pair(s2, 6, x1T, evict="dve")]
                window(s2, ctxt2, extra2)

                ctxT2 = p_ctxT.tile([P, KC, TP], BF16, tag="ctxT")
                for tb in range(TB):
                    tr_tb(ctxt2, ctxT2, tb, "dve")
            # attention stack closed: kv/weights/ets SBUF freed

            # ---------------- FFN weight prefetch ----------------
            p_hT = ctx.enter_context(tc.tile_pool(name="p_hT", bufs=1))
            hT = p_hT.tile([P, FFC, NQ], BF16)
            wir = win_d.rearrange("(c p) m -> p c m", p=P)
            wor = wout_d.rearrange("(c p) m -> p c m", p=P)
            with tc.tile_pool(name="p_wit", bufs=2) as p_wit, \
                    tc.tile_pool(name="p_wot", bufs=2) as p_wot:
                NWQ = 4  # w_in quarter chunks, ring of 2
                QW = FF // NWQ
                wits = []
                for q in range(2):
                    w = p_wit.tile([P, KC, QW], BF16, tag="wit")
                    nc.sync.dma_start(w, wir[:, :, q * QW:(q + 1) * QW])
                    wits.append(w)
                wots = []
                for oh in range(2):
                    w = p_wot.tile([P, FFC, 512], BF16, tag="wot")
                    nc.sync.dma_start(w, wor[:, :, oh * 512:(oh + 1) * 512])
                    wots.append(w)

                # ---------------- s2 out-projection ----------------
                res2 = p_res.tile([P, TB, D], BF16, tag="res")
                x2 = p_res.tile([P, TB, D], BF16, tag="res")
                x2T = p_xT.tile([P, KC, TP], BF16, tag="xT")
                out_stage(ctxT2, wo2, x1, res2, x2, x2T, "act")

                # ---------------- FFN hidden ----------------
                FPQ = QW // (2 * P)  # ffc-pairs per quarter
                for fp in range(FFC // 2):
                    q = fp // FPQ
                    if q >= 2 and fp % FPQ == 0:
                        w = p_wit.tile([P, KC, QW], BF16, tag="wit")
                        nc.sync.dma_start(
                            w, wir[:, :, q * QW:(q + 1) * QW])
                        wits.append(w)
                    hh = pp_sc.tile([P, 2, NQ], F32, tag="ps", name="ps")
                    wt = wits[q]
                    base = q * QW
                    for j in range(2):
                        c0 = (2 * fp + j) * P - base
                        for kc in range(KC):
                            nc.tensor.matmul(
                                hh[:, j, :], wt[:, kc, c0:c0 + P],
                                x2T[:, kc, :],
                                start=(kc == 0), stop=(kc == KC - 1))
                    nc.scalar.activation(hT[:, 2 * fp:2 * fp + 2, :], hh,
                                         AF.Relu)
                res3 = p_res.tile([P, TB, D], BF16, tag="res")
                outr = out_d.rearrange("(tb p) d -> p tb d", p=P)
                for oh in range(2):
                    for tbp in range(TB // 2):
                        po = pp_sc.tile([P, 2, 512], F32, tag="ps", name="ps")
                        for ffc in range(FFC):
                            for i in range(2):
                                tb = 2 * tbp + i
                                nc.tensor.matmul(
                                    po[:, i, :],
                                    hT[:, ffc, tb * P:(tb + 1) * P],
                                    wots[oh][:, ffc, :],
                                    start=(ffc == 0), stop=(ffc == FFC - 1))
                        for i in range(2):
                            tb = 2 * tbp + i
                            nc.vector.tensor_add(
                                res3[:, tb, oh * 512:(oh + 1) * 512],
                                po[:, i, :],
                                x2[:, tb, oh * 512:(oh + 1) * 512])
                            if oh == 1:
                                xot = p_resf.tile([P, 1, D], F32, tag="resf",
                                                  name="xot")
                                # per-tb LN into a [P,1,D] staging tile
                                stv = p_stat.tile([P, 2, 6], F32, tag="lnst")
                                for g in range(2):
                                    nc.vector.bn_stats(
                                        stv[:, g, :],
                                        res3[:, tb, g * 512:(g + 1) * 512])
                                mv = p_stat.tile([P, 2], F32, tag="lnmv")
                                nc.vector.bn_aggr(mv, stv)
                                std = p_stat.tile([P, 1], F32, tag="lnstd")
                                nc.scalar.activation(std, mv[:, 1:2],
                                                     AF.Sqrt, bias=eps_t)
                                rstd = p_stat.tile([P, 1], F32, tag="lnrstd")
                                nc.vector.reciprocal(rstd, std)
                                nc.vector.tensor_scalar(
                                    out=xot[:, 0, :], in0=res3[:, tb, :],
                                    scalar1=mv[:, 0:1], scalar2=rstd,
                                    op0=ALU.subtract, op1=ALU.mult)
                                nc.sync.dma_start(outr[:, tb, :],
                                                  xot[:, 0, :])

    nc.compile()
    return nc


# ---------------------------------------------------------------------------
# host side
# ---------------------------------------------------------------------------

_NC_CACHE = {}

MM_KEY = ("v4",)


def _get_nc(key=MM_KEY):
    if key not in _NC_CACHE:
        _NC_CACHE[key] = build_decoder_nc()
    return _NC_CACHE[key]


def _numpy_reference(x, enc_out, src_mask, tgt_mask, wq1, bq1, wkv1, bkv1,
                     wo1, bo1, wq2, bq2, wkv2, bkv2, wo2, bo2, w_in, b_in,
                     w_out, b_out, g0, be0, g1, be1, g2, be2):
    """Pure-numpy fallback (exact reference semantics)."""
    H, D = 16, 1024

    def ln(x, g, b):
        m = x.mean(-1, keepdims=True)
        v = ((x - m) ** 2).mean(-1, keepdims=True)
        return (x - m) / np.sqrt(v + LN_EPS) * g + b

    def attn(q_in, mem, mask, wq, bq, wkv, bkv, wo, bo):
        B, T, _ = q_in.shape
        S = mem.shape[1]
        dhl = D // H
        q = (q_in @ wq + bq).reshape(B, T, H, dhl) * (dhl ** -0.5)
        k, v = np.split(mem @ wkv + bkv, 2, axis=-1)
        k = k.reshape(B, S, H, dhl)
        v = v.reshape(B, S, H, dhl)
        sc = np.einsum('bthd,bshd->bhts', q, k)
        sc = np.where(mask[:, None, :, :], -1e20, sc)
        sc = sc - sc.max(-1, keepdims=True)
        w = np.exp(sc)
        w = w / w.sum(-1, keepdims=True)
        ctx = np.einsum('bhts,bshd->bthd', w, v).reshape(B, T, D)
        return ctx @ wo + bo

    y = attn(x, x, tgt_mask, wq1, bq1, wkv1, bkv1, wo1, bo1)
    x1 = ln(x + y, g0, be0)
    y = attn(x1, enc_out, src_mask, wq2, bq2, wkv2, bkv2, wo2, bo2)
    x2 = ln(x1 + y, g1, be1)
    y = np.maximum(x2 @ w_in + b_in, 0.0) @ w_out + b_out
    return ln(x2 + y, g2, be2)


def kernel(x, enc_out, src_mask, tgt_mask, wq1, bq1, wkv1, bkv1, wo1, bo1,
           wq2, bq2, wkv2, bkv2, wo2, bo2, w_in, b_in, w_out, b_out,
           g0, be0, g1, be1, g2, be2, _trace=False):
    x = np.asarray(x)
    args = dict(x=x, enc_out=np.asarray(enc_out),
                src_mask=np.asarray(src_mask), tgt_mask=np.asarray(tgt_mask),
                wq1=np.asarray(wq1), bq1=np.asarray(bq1),
                wkv1=np.asarray(wkv1), bkv1=np.asarray(bkv1),
                wo1=np.asarray(wo1), bo1=np.asarray(bo1),
                wq2=np.asarray(wq2), bq2=np.asarray(bq2),
                wkv2=np.asarray(wkv2), bkv2=np.asarray(bkv2),
                wo2=np.asarray(wo2), bo2=np.asarray(bo2),
                w_in=np.asarray(w_in), b_in=np.asarray(b_in),
                w_out=np.asarray(w_out), b_out=np.asarray(b_out),
                g0=np.asarray(g0), be0=np.asarray(be0),
                g1=np.asarray(g1), be1=np.asarray(be1),
                g2=np.asarray(g2), be2=np.asarray(be2))

    # the hardware kernel folds out zero biases / unit gains (true for this
    # problem's setup_inputs); anything else falls back to exact numpy.
    zeros = [args[k] for k in ("bq1", "bkv1", "bo1", "bq2", "bkv2", "bo2",
                               "b_in", "b_out", "be0", "be1", "be2")]
    ones = [args["g0"], args["g1"], args["g2"]]
    if any(np.any(z != 0) for z in zeros) or any(np.any(g != 1) for g in ones):
        res = _numpy_reference(**args)
        return res.astype(np.float32), x

    B, T, D = x.shape
    TP = T // 2
    bf = ml_dtypes.bfloat16
    f8 = ml_dtypes.float8_e4m3

    def cbf(a):
        return np.ascontiguousarray(a.astype(bf))

    def c8(a):
        return np.ascontiguousarray(a.astype(f8))

    wq1b = c8(args["wq1"] * np.float32(4.0))
    wq2b = c8(args["wq2"] * np.float32(4.0))
    wkv1b = c8(args["wkv1"] * np.float32(32.0))
    wkv2b = c8(args["wkv2"] * np.float32(32.0))
    wo1b = cbf(args["wo1"])
    wo2b = cbf(args["wo2"])
    w_inb = cbf(args["w_in"])
    w_outb = cbf(args["w_out"])

    in_maps = []
    for core in range(8):
        b, half = divmod(core, 2)
        t0 = half * TP
        xb = args["x"][b]
        xs = xb[t0:t0 + TP]
        in_maps.append({
            "xqT": c8(xs.T),
            "xq": cbf(xs),
            "xfT": c8(xb.T),
            "encT": c8(args["enc_out"][b].T),
            "k1T": cbf((~args["tgt_mask"][b, t0:t0 + TP]).T
                       .astype(np.float32)),
            "k2T": cbf((~args["src_mask"][b, t0:t0 + TP]).T
                       .astype(np.float32)),
            "wq1": wq1b,
            "wkv1": wkv1b,
            "wo1": wo1b,
            "wq2": wq2b,
            "wkv2": wkv2b,
            "wo2": wo2b,
            "w_in": w_inb,
            "w_out": w_outb,
        })

    nc = _get_nc(MM_KEY)
    res = run_bass_kernel_spmd(nc, in_maps, core_ids=list(range(8)),
                               trace=_trace)
    outp = np.empty((B, T, D), np.float32)
    for core in range(8):
        b, half = divmod(core, 2)
        outp[b, half * TP:(half + 1) * TP] = res.results[core]["out"]
    if _trace:
        kernel.last_results = res
    return outp, x


# revision 15
# speedup vs baseline: 1.1198x; 1.1172x over previous
"""Trainium2 Bass kernel for nn_DecoderLayer (self-attn + cross-attn + FFN).

Sharding: 8 cores = (batch b in 0..3) x (query-half in 0..1). Each core
computes 512 query tokens of one batch element end-to-end; K/V projections
over the full source sequence are duplicated across the two halves of a
batch element, so no collectives are needed.

v4 design (cost-model driven):
  - fp8e4m3 + DoubleRow (2 contraction k-tiles per instruction, 0.5
    cycles/row) for every D-contraction projection (q/k/v for both
    attentions). Weights are pre-scaled host-side (wkv*32, wq*4) so fp8
    operands sit in the normal range; the 2^-10 descale folds into the
    exp's scale and the softmax-denominator ones-column (=32).
  - fp8 scores (k,q evicted as fp8; softmax averaging absorbs the ~4%
    quantization noise). exp -> bf16 probabilities; multiplicative keep
    mask on DVE in the 2-byte 2x mode; bf16 ctx matmuls.
  - out-projections and the FFN stay bf16 (their noise lands directly on
    the output with no averaging, fp8 would blow the error budget).
  - stage-2's K/V projections are emitted INSIDE stage-1's attention
    window: the window is paced by the ACT engine's exps, so the PE would
    otherwise idle there. Engine streams are in-order, so chunks are
    interleaved between each head-pair's scores and ctx.
  - psum evictions spread across engines by phase: ACT evicts during
    projection phases (idle then), gpsimd during attention windows (ACT
    is busy with exps), DVE takes masks/normalize/residual/LN.
  - out-projection + LN + transpose run per-token-block so the tail
    pipelines; FFN weights prefetch during the preceding phase.

Self-contained: hardcodes all shapes; no sibling imports.
"""

import numpy as np
import ml_dtypes
from contextlib import ExitStack

import concourse.bass as bass
import concourse.tile as tile
from concourse import bacc, mybir
from concourse.bass_utils import run_bass_kernel_spmd
from concourse.masks import make_identity

P = 128
LN_EPS = 1e-5

F32 = mybir.dt.float32
BF16 = mybir.dt.bfloat16
FP8 = mybir.dt.float8e4

AF = mybir.ActivationFunctionType
ALU = mybir.AluOpType
DR = mybir.MatmulPerfMode.DoubleRow

EXP_SCALE = 1.0 / 1024.0  # descale the (32q).(32k) fp8 score accumulation


def build_decoder_nc(D=1024, S=1024, TP=512, H=16, FF=4096):
    dh = 64
    KC = D // P      # contraction chunks over D
    KC2 = KC // 2    # DoubleRow chunk pairs
    SB = S // P      # source blocks
    TB = TP // P     # query-token blocks
    NQ = TP
    FFC = FF // P
    HPV = 512 // dh  # heads per v-proj half
    assert NQ <= 512 and H * dh == D

    nc = bacc.Bacc("TRN2", target_bir_lowering=False, debug=False)

    def din(name, shape, dt):
        return nc.dram_tensor(name, shape, dt, kind="ExternalInput").ap()

    xqT_d = din("xqT", [D, TP], FP8)
    xq_d = din("xq", [TP, D], BF16)
    xfT_d = din("xfT", [D, S], FP8)
    encT_d = din("encT", [D, S], FP8)
    k1T_d = din("k1T", [S, TP], BF16)
    k2T_d = din("k2T", [S, TP], BF16)
    wq1_d = din("wq1", [D, D], FP8)        # pre-scaled 4x (incl dh**-0.5)
    wkv1_d = din("wkv1", [D, 2 * D], FP8)  # pre-scaled 32x
    wo1_d = din("wo1", [D, D], BF16)
    wq2_d = din("wq2", [D, D], FP8)
    wkv2_d = din("wkv2", [D, 2 * D], FP8)
    wo2_d = din("wo2", [D, D], BF16)
    win_d = din("w_in", [D, FF], BF16)
    wout_d = din("w_out", [FF, D], BF16)
    out_d = nc.dram_tensor("out", [TP, D], F32, kind="ExternalOutput").ap()

    with tile.TileContext(nc) as tc:
        with ExitStack() as ctx:
            # ---- bottom persistent pools (alive through FFN) ----
            consts = ctx.enter_context(tc.tile_pool(name="consts", bufs=1))
            p_stat = ctx.enter_context(tc.tile_pool(name="p_stat", bufs=10))
            p_res = ctx.enter_context(tc.tile_pool(name="p_res", bufs=3))
            p_resf = ctx.enter_context(tc.tile_pool(name="p_resf", bufs=2))
            p_xT = ctx.enter_context(tc.tile_pool(name="p_xT", bufs=1))
            p_ctxT = ctx.enter_context(tc.tile_pool(name="p_ctxT", bufs=1))
            p_wo = ctx.enter_context(tc.tile_pool(name="p_wo", bufs=1))
            pp_sc = ctx.enter_context(
                tc.tile_pool(name="pp_sc", bufs=2, space="PSUM"))
            pp_ctx = ctx.enter_context(
                tc.tile_pool(name="pp_ctx", bufs=2, space="PSUM"))
            pp_tr = ctx.enter_context(
                tc.tile_pool(name="pp_tr", bufs=2, space="PSUM"))

            ident = consts.tile([P, P], BF16)
            make_identity(nc, ident)
            eps_t = consts.tile([P, 1], F32)
            nc.vector.memset(eps_t, LN_EPS)

            def dr_chain(ps, lhs_t, l0, l1, rhs_t, r0, r1):
                """4 DoubleRow matmuls accumulating lhsT.T@rhs over D."""
                for kc2 in range(KC2):
                    k = 2 * kc2
                    nc.tensor.matmul(
                        ps, lhs_t[:, k:k + 2, l0:l1], rhs_t[:, k:k + 2, r0:r1],
                        start=(kc2 == 0), stop=(kc2 == KC2 - 1), perf_mode=DR)

            def ln_tb(res, xout, tb):
                st = p_stat.tile([P, 2, 6], F32, tag="lnst")
                for g in range(2):
                    nc.vector.bn_stats(st[:, g, :],
                                       res[:, tb, g * 512:(g + 1) * 512])
                mv = p_stat.tile([P, 2], F32, tag="lnmv")
                nc.vector.bn_aggr(mv, st)
                std = p_stat.tile([P, 1], F32, tag="lnstd")
                nc.scalar.activation(std, mv[:, 1:2], AF.Sqrt, bias=eps_t)
                rstd = p_stat.tile([P, 1], F32, tag="lnrstd")
                nc.vector.reciprocal(rstd, std)
                nc.vector.tensor_scalar(
                    out=xout[:, tb, :], in0=res[:, tb, :],
                    scalar1=mv[:, 0:1], scalar2=rstd,
                    op0=ALU.subtract, op1=ALU.mult)

            def tr_tb(src, dstT, tb, evict):
                for fc in range(KC):
                    ps = pp_tr.tile([P, P], BF16)
                    nc.tensor.transpose(
                        ps, src[:, tb, fc * P:(fc + 1) * P], ident)
                    dst = dstT[:, fc, tb * P:(tb + 1) * P]
                    if evict == "act":
                        nc.scalar.copy(dst, ps)
                    else:
                        nc.vector.tensor_copy(dst, ps)

            # ================= attention stack =================
            with ExitStack() as attn:
                p_kT = attn.enter_context(tc.tile_pool(name="p_kT", bufs=2))
                p_vt = attn.enter_context(tc.tile_pool(name="p_vt", bufs=2))
                p_qT = attn.enter_context(tc.tile_pool(name="p_qT", bufs=1))
                p_mT = attn.enter_context(tc.tile_pool(name="p_mT", bufs=1))
                p_kvs = attn.enter_context(tc.tile_pool(name="p_kvs", bufs=1))
                p_wkv = attn.enter_context(tc.tile_pool(name="p_wkv", bufs=1))
                p_wq = attn.enter_context(tc.tile_pool(name="p_wq", bufs=1))
                p_rsrc = attn.enter_context(
                    tc.tile_pool(name="p_rsrc", bufs=1))
                p_ets = attn.enter_context(tc.tile_pool(name="p_ets", bufs=16))

                class Stage:
                    pass

                def new_stage(kvT_dram, wkv_dram, mT_dram):
                    st = Stage()
                    st.kT = p_kT.tile([P, KC, S], FP8, tag="kT")
                    st.vt = p_vt.tile([P, SB, H, dh + 1], BF16, tag="vt")
                    st.qT = p_qT.tile([P, KC, NQ], FP8, tag="qT")
                    st.mT = p_mT.tile([P, SB, NQ], BF16, tag="mT")
                    st.kvs = p_kvs.tile([P, KC, S], FP8, tag="kvs")
                    st.wkv = p_wkv.tile([P, KC, 2 * D], FP8, tag="wkv")
                    st.ets = {}
                    nc.vector.memset(st.vt[:, :, :, dh:dh + 1], 32.0)
                    nc.sync.dma_start(
                        st.kvs, kvT_dram.rearrange("(c p) s -> p c s", p=P))
                    wr = wkv_dram.rearrange("(c p) m -> p c m", p=P)
                    nc.sync.dma_start(st.wkv[:, :, 0:D], wr[:, :, 0:D])
                    nc.sync.dma_start(st.wkv[:, :, D:2 * D], wr[:, :, D:2 * D])
                    nc.sync.dma_start(
                        st.mT, mT_dram.rearrange("(sb p) t -> p sb t", p=P))
                    return st

                def kT_chunk(st, of, evict):
                    kk = pp_sc.tile([P, 2, 512], F32, tag="ps", name="ps")
                    for sh in range(2):
                        dr_chain(kk[:, sh, :], st.wkv, of * P, (of + 1) * P,
                                 st.kvs, sh * 512, (sh + 1) * 512)
                    src = kk.rearrange("p a b -> p (a b)")
                    dst = st.kT[:, of, :]
                    if evict == "act":
                        nc.scalar.copy(dst, src)
                    else:
                        nc.vector.tensor_copy(dst, src)

                def v_chunk(st, vh, sbp, evict):
                    vv = pp_sc.tile([P, 2, 512], F32, tag="ps", name="ps")
                    for i in range(2):
                        sb = 2 * sbp + i
                        dr_chain(vv[:, i, :], st.kvs, sb * P, (sb + 1) * P,
                                 st.wkv, D + vh * 512, D + (vh + 1) * 512)
                    src = vv.rearrange("p a (h d) -> p a h d", d=dh)
                    dst = st.vt[:, 2 * sbp:2 * sbp + 2,
                                vh * HPV:(vh + 1) * HPV, 0:dh]
                    if evict == "act":
                        nc.scalar.copy(dst, src)
                    else:
                        nc.vector.tensor_copy(dst, src)

                def qT_pair(st, of, qsrc, evict):
                    qq = pp_sc.tile([P, 2, NQ], F32, tag="ps", name="ps")
                    for j in range(2):
                        dr_chain(qq[:, j, :], st.wq, (of + j) * P,
                                 (of + j + 1) * P, qsrc, 0, NQ)
                    dst = st.qT[:, of:of + 2, :]
                    if evict == "act":
                        nc.scalar.copy(dst, qq)
                    else:
                        nc.vector.tensor_copy(dst, qq)

                def sc_tile(st, h, sbp):
                    kc_h, ko = divmod(h * dh, P)
                    sc = pp_sc.tile([P, 2, NQ], F32, tag="ps", name="ps")
                    for i in range(2):
                        sb = 2 * sbp + i
                        nc.tensor.matmul(
                            sc[:, i, :],
                            st.kT[ko:ko + dh, kc_h, sb * P:(sb + 1) * P],
                            st.qT[ko:ko + dh, kc_h, :],
                            start=True, stop=True)
                    et = p_ets.tile([P, 2, NQ], BF16, name="et")
                    nc.scalar.activation(et, sc, AF.Exp, scale=EXP_SCALE)
                    eng = (nc.gpsimd if (h * 4 + sbp) % 3 == 0
                           else nc.vector)
                    eng.tensor_mul(et, et, st.mT[:, 2 * sbp:2 * sbp + 2, :])
                    st.ets[(h, sbp)] = et

                def ctx_chain(st, ctxt, h, tb):
                    psc = pp_ctx.tile([P, dh + 1], F32, name="psc")
                    n = 0
                    for sbp in range(SB // 2):
                        for i in range(2):
                            nc.tensor.matmul(
                                psc,
                                st.ets[(h, sbp)][:, i, tb * P:(tb + 1) * P],
                                st.vt[:, 2 * sbp + i, h, :],
                                start=(n == 0), stop=(n == SB - 1))
                            n += 1
                    rec = p_stat.tile([P, 1], F32, tag="rec", name="rec")
                    nc.vector.reciprocal(rec, psc[:, dh:dh + 1])
                    nc.vector.tensor_scalar_mul(
                        ctxt[:, tb, h * dh:(h + 1) * dh],
                        in0=psc[:, 0:dh], scalar1=rec)

                def window(st, ctxt, extra_per_hp):
                    """Attention window: 8 slots per head-pair; each slot
                    emits [prev ctx chain][extra work][scores tile] so the
                    PE fills the exp-paced psum-ring waits."""
                    prev = None
                    for hpi, hp in enumerate(range(0, H, 2)):
                        sc_list = [(h, sbp) for h in (hp, hp + 1)
                                   for sbp in range(SB // 2)]
                        cx_list = ([] if prev is None else
                                   [(h, tb) for h in (prev, prev + 1)
                                    for tb in range(TB)])
                        extra = list(extra_per_hp[hpi])
                        for k in range(8):
                            if k < len(cx_list):
                                ctx_chain(st, ctxt, *cx_list[k])
                            if k >= 3 and extra:
                                extra.pop(0)()
                            sc_tile(st, *sc_list[k])
                        for em in extra:
                            em()
                        prev = hp
                    for h in (prev, prev + 1):
                        for tb in range(TB):
                            ctx_chain(st, ctxt, h, tb)

                def out_stage(ctxT, wot, rsrc, res, xo, xT, tr_evict):
                    for tb in range(TB):
                        po = pp_sc.tile([P, 2, 512], F32, tag="ps", name="ps")
                        for fc in range(KC):
                            for oh in range(2):
                                nc.tensor.matmul(
                                    po[:, oh, :],
                                    ctxT[:, fc, tb * P:(tb + 1) * P],
                                    wot[:, fc, oh * 512:(oh + 1) * 512],
                                    start=(fc == 0), stop=(fc == KC - 1))
                        nc.vector.tensor_add(
                            res[:, tb, :], po.rearrange("p a b -> p (a b)"),
                            rsrc[:, tb, :])
                        ln_tb(res, xo, tb)
                        if tb >= 1:
                            tr_tb(xo, xT, tb - 1, tr_evict)
                    tr_tb(xo, xT, TB - 1, tr_evict)

                # ---------------- DMA prefetch + stage tiles ----------
                s1 = new_stage(xfT_d, wkv1_d, k1T_d)
                s1.wq = p_wq.tile([P, KC, D], FP8, tag="wq")
                nc.sync.dma_start(
                    s1.wq, wq1_d.rearrange("(c p) m -> p c m", p=P))


                s2 = new_stage(encT_d, wkv2_d, k2T_d)
                s2.wq = p_wq.tile([P, KC, D], FP8, tag="wq")
                nc.sync.dma_start(
                    s2.wq, wq2_d.rearrange("(c p) m -> p c m", p=P))

                wo1 = p_wo.tile([P, KC, D], BF16, tag="wo")
                nc.sync.dma_start(
                    wo1, wo1_d.rearrange("(c p) m -> p c m", p=P))
                rsrc1 = p_rsrc.tile([P, TB, D], BF16, tag="rsrc")
                nc.sync.dma_start(
                    rsrc1, xq_d.rearrange("(tb p) d -> p tb d", p=P))

                # ---------------- s1 projections ----------------
                with tc.tile_pool(name="p_qsrc", bufs=1) as p_qsrc:
                    qsrc = p_qsrc.tile([P, KC, NQ], FP8, tag="qsrc")
                    nc.sync.dma_start(
                        qsrc, xqT_d.rearrange("(c p) t -> p c t", p=P))
                    for of in range(KC):
                        kT_chunk(s1, of, evict="act")
                    for vh in range(2):
                        for sbp in range(SB // 2):
                            v_chunk(s1, vh, sbp, evict="dve")
                    for of in range(0, KC, 2):
                        qT_pair(s1, of, qsrc, evict="dve")

                # ---------------- window 1: s1 attention + s2 kv ------
                ctxt1 = p_res.tile([P, TB, D], BF16, tag="res")
                kv2_work = [lambda of=of: kT_chunk(s2, of, evict="dve")
                            for of in range(KC)]
                kv2_work += [lambda vh=vh, sbp=sbp:
                             v_chunk(s2, vh, sbp, evict="dve")
                             for vh in range(2) for sbp in range(SB // 2)]
                kv2_counts = [0, 2, 2, 2, 2, 2, 3, 3]
                extra1 = []
                wi = 0
                for c in kv2_counts:
                    extra1.append(kv2_work[wi:wi + c])
                    wi += c
                window(s1, ctxt1, extra1)

                # ---------------- s1 out-projection ----------------
                ctxT1 = p_ctxT.tile([P, KC, TP], BF16, tag="ctxT")
                for tb in range(TB):
                    tr_tb(ctxt1, ctxT1, tb, "dve")
                res1 = p_res.tile([P, TB, D], BF16, tag="res")
                x1 = p_res.tile([P, TB, D], BF16, tag="res")
                x1T = p_xT.tile([P, KC, TP], FP8, tag="xT")
                out_stage(ctxT1, wo1, rsrc1, res1, x1, x1T, "act")

                wo2 = p_wo.tile([P, KC, D], BF16, tag="wo")
                nc.sync.dma_start(
                    wo2, wo2_d.rearrange("(c p) m -> p c m", p=P))

                # ---------------- window 2: s2 attention ----------------
                ctxt2 = p_res.tile([P, TB, D], BF16, tag="res")
                qT_pair(s2, 0, x1T, evict="dve")
                qT_pair(s2, 2, x1T, evict="dve")
                extra2 = [[] for _ in range(8)]
                extra2[1] = [lambda: qT_pair(s2, 4, x1T, evict="dve")]
                extra2[3] = [lambda: qT_pair(s2, 6, x1T, evict="dve")]
                window(s2, ctxt2, extra2)

                ctxT2 = p_ctxT.tile([P, KC, TP], BF16, tag="ctxT")
                for tb in range(TB):
                    tr_tb(ctxt2, ctxT2, tb, "dve")
            # attention stack closed: kv/weights/ets SBUF freed

            # ---------------- FFN weight prefetch ----------------
            p_hT = ctx.enter_context(tc.tile_pool(name="p_hT", bufs=1))
            hT = p_hT.tile([P, FFC, NQ], BF16)
            wir = win_d.rearrange("(c p) m -> p c m", p=P)
            wor = wout_d.rearrange("(c p) m -> p c m", p=P)
            with tc.tile_pool(name="p_wit", bufs=2) as p_wit, \
                    tc.tile_pool(name="p_wot", bufs=2) as p_wot:
                NWQ = 4  # w_in quarter chunks, ring of 2
                QW = FF // NWQ
                wits = []
                for q in range(2):
                    w = p_wit.tile([P, KC, QW], BF16, tag="wit")
                    nc.sync.dma_start(w, wir[:, :, q * QW:(q + 1) * QW])
                    wits.append(w)
                wots = []
                for oh in range(2):
                    w = p_wot.tile([P, FFC, 512], BF16, tag="wot")
                    nc.sync.dma_start(w, wor[:, :, oh * 512:(oh + 1) * 512])
                    wots.append(w)

                # ---------------- s2 out-projection ----------------
                res2 = p_res.tile([P, TB, D], BF16, tag="res")
                x2 = p_res.tile([P, TB, D], BF16, tag="res")
                x2T = p_xT.tile([P, KC, TP], BF16, tag="xT")
                out_stage(ctxT2, wo2, x1, res2, x2, x2T, "act")

                # ---------------- FFN hidden ----------------
                FPQ = QW // (2 * P)  # ffc-pairs per quarter
                for fp in range(FFC // 2):
                    q = fp // FPQ
                    if q >= 2 and fp % FPQ == 0:
                        w = p_wit.tile([P, KC, QW], BF16, tag="wit")
                        nc.sync.dma_start(
                            w, wir[:, :, q * QW:(q + 1) * QW])
                        wits.append(w)
                    hh = pp_sc.tile([P, 2, NQ], F32, tag="ps", name="ps")
                    wt = wits[q]
                    base = q * QW
                    for j in range(2):
                        c0 = (2 * fp + j) * P - base
                        for kc in range(KC):
                            nc.tensor.matmul(
                                hh[:, j, :], wt[:, kc, c0:c0 + P],
                                x2T[:, kc, :],
                                start=(kc == 0), stop=(kc == KC - 1))
                    nc.scalar.activation(hT[:, 2 * fp:2 * fp + 2, :], hh,
                                         AF.Relu)
                res3 = p_res.tile([P, TB, D], BF16, tag="res")
                outr = out_d.rearrange("(tb p) d -> p tb d", p=P)
                for oh in range(2):
                    for tbp in range(TB // 2):
                        po = pp_sc.tile([P, 2, 512], F32, tag="ps", name="ps")
                        for ffc in range(FFC):
                            for i in range(2):
                                tb = 2 * tbp + i
                                nc.tensor.matmul(
                                    po[:, i, :],
                                    hT[:, ffc, tb * P:(tb + 1) * P],
                                    wots[oh][:, ffc, :],
                                    start=(ffc == 0), stop=(ffc == FFC - 1))
                        for i in range(2):
                            tb = 2 * tbp + i
                            nc.vector.tensor_add(
                                res3[:, tb, oh * 512:(oh + 1) * 512],
                                po[:, i, :],
                                x2[:, tb, oh * 512:(oh + 1) * 512])
                            if oh == 1:
                                xot = p_resf.tile([P, 1, D], F32, tag="resf",
                                                  name="xot")
                                # per-tb LN into a [P,1,D] staging tile
                                stv = p_stat.tile([P, 2, 6], F32, tag="lnst")
                                for g in range(2):
                                    nc.vector.bn_stats(
                                        stv[:, g, :],
                                        res3[:, tb, g * 512:(g + 1) * 512])
                                mv = p_stat.tile([P, 2], F32, tag="lnmv")
                                nc.vector.bn_aggr(mv, stv)
                                std = p_stat.tile([P, 1], F32, tag="lnstd")
                                nc.scalar.activation(std, mv[:, 1:2],
                                                     AF.Sqrt, bias=eps_t)
                                rstd = p_stat.tile([P, 1], F32, tag="lnrstd")
                                nc.vector.reciprocal(rstd, std)
                                nc.vector.tensor_scalar(
                                    out=xot[:, 0, :], in0=res3[:, tb, :],
                                    scalar1=mv[:, 0:1], scalar2=rstd,
                                    op0=ALU.subtract, op1=ALU.mult)
                                nc.sync.dma_start(outr[:, tb, :],
                                                  xot[:, 0, :])

    nc.compile()
    return nc


# ---------------------------------------------------------------------------
# host side
# ---------------------------------------------------------------------------

_NC_CACHE = {}

MM_KEY = ("v4",)


def _get_nc(key=MM_KEY):
    if key not in _NC_CACHE:
        _NC_CACHE[key] = build_decoder_nc()
    return _NC_CACHE[key]


def _numpy_reference(x, enc_out, src_mask, tgt_mask, wq1, bq1, wkv1, bkv1,
                     wo1, bo1, wq2, bq2, wkv2, bkv2, wo2, bo2, w_in, b_in,
                     w_out, b_out, g0, be0, g1, be1, g2, be2):
    """Pure-numpy fallback (exact reference semantics)."""
    H, D = 16, 1024

    def ln(x, g, b):
        m = x.mean(-1, keepdims=True)
        v = ((x - m) ** 2).mean(-1, keepdims=True)
        return (x - m) / np.sqrt(v + LN_EPS) * g + b

    def attn(q_in, mem, mask, wq, bq, wkv, bkv, wo, bo):
        B, T, _ = q_in.shape
        S = mem.shape[1]
        dhl = D // H
        q = (q_in @ wq + bq).reshape(B, T, H, dhl) * (dhl ** -0.5)
        k, v = np.split(mem @ wkv + bkv, 2, axis=-1)
        k = k.reshape(B, S, H, dhl)
        v = v.reshape(B, S, H, dhl)
        sc = np.einsum('bthd,bshd->bhts', q, k)
        sc = np.where(mask[:, None, :, :], -1e20, sc)
        sc = sc - sc.max(-1, keepdims=True)
        w = np.exp(sc)
        w = w / w.sum(-1, keepdims=True)
        ctx = np.einsum('bhts,bshd->bthd', w, v).reshape(B, T, D)
        return ctx @ wo + bo

    y = attn(x, x, tgt_mask, wq1, bq1, wkv1, bkv1, wo1, bo1)
    x1 = ln(x + y, g0, be0)
    y = attn(x1, enc_out, src_mask, wq2, bq2, wkv2, bkv2, wo2, bo2)
    x2 = ln(x1 + y, g1, be1)
    y = np.maximum(x2 @ w_in + b_in, 0.0) @ w_out + b_out
    return ln(x2 + y, g2, be2)


def kernel(x, enc_out, src_mask, tgt_mask, wq1, bq1, wkv1, bkv1, wo1, bo1,
           wq2, bq2, wkv2, bkv2, wo2, bo2, w_in, b_in, w_out, b_out,
           g0, be0, g1, be1, g2, be2, _trace=False):
    x = np.asarray(x)
    args = dict(x=x, enc_out=np.asarray(enc_out),
                src_mask=np.asarray(src_mask), tgt_mask=np.asarray(tgt_mask),
                wq1=np.asarray(wq1), bq1=np.asarray(bq1),
                wkv1=np.asarray(wkv1), bkv1=np.asarray(bkv1),
                wo1=np.asarray(wo1), bo1=np.asarray(bo1),
                wq2=np.asarray(wq2), bq2=np.asarray(bq2),
                wkv2=np.asarray(wkv2), bkv2=np.asarray(bkv2),
                wo2=np.asarray(wo2), bo2=np.asarray(bo2),
                w_in=np.asarray(w_in), b_in=np.asarray(b_in),
                w_out=np.asarray(w_out), b_out=np.asarray(b_out),
                g0=np.asarray(g0), be0=np.asarray(be0),
                g1=np.asarray(g1), be1=np.asarray(be1),
                g2=np.asarray(g2), be2=np.asarray(be2))

    # the hardware kernel folds out zero biases / unit gains (true for this
    # problem's setup_inputs); anything else falls back to exact numpy.
    zeros = [args[k] for k in ("bq1", "bkv1", "bo1", "bq2", "bkv2", "bo2",
                               "b_in", "b_out", "be0", "be1", "be2")]
    ones = [args["g0"], args["g1"], args["g2"]]
    if any(np.any(z != 0) for z in zeros) or any(np.any(g != 1) for g in ones):
        res = _numpy_reference(**args)
        return res.astype(np.float32), x

    B, T, D = x.shape
    TP = T // 2
    bf = ml_dtypes.bfloat16
    f8 = ml_dtypes.float8_e4m3

    def cbf(a):
        return np.ascontiguousarray(a.astype(bf))

    def c8(a):
        return np.ascontiguousarray(a.astype(f8))

    wq1b = c8(args["wq1"] * np.float32(4.0))
    wq2b = c8(args["wq2"] * np.float32(4.0))
    wkv1b = c8(args["wkv1"] * np.float32(32.0))
    wkv2b = c8(args["wkv2"] * np.float32(32.0))
    wo1b = cbf(args["wo1"])
    wo2b = cbf(args["wo2"])
    w_inb = cbf(args["w_in"])
    w_outb = cbf(args["w_out"])

    in_maps = []
    for core in range(8):
        b, half = divmod(core, 2)
        t0 = half * TP
        xb = args["x"][b]
        xs = xb[t0:t0 + TP]
        in_maps.append({
            "xqT": c8(xs.T),
            "xq": cbf(xs),
            "xfT": c8(xb.T),
            "encT": c8(args["enc_out"][b].T),
            "k1T": cbf((~args["tgt_mask"][b, t0:t0 + TP]).T
                       .astype(np.float32)),
            "k2T": cbf((~args["src_mask"][b, t0:t0 + TP]).T
                       .astype(np.float32)),
            "wq1": wq1b,
            "wkv1": wkv1b,
            "wo1": wo1b,
            "wq2": wq2b,
            "wkv2": wkv2b,
            "wo2": wo2b,
            "w_in": w_inb,
            "w_out": w_outb,
        })

    nc = _get_nc(MM_KEY)
    res = run_bass_kernel_spmd(nc, in_maps, core_ids=list(range(8)),
                               trace=_trace)
    outp = np.empty((B, T, D), np.float32)
    for core in range(8):
        b, half = divmod(core, 2)
        outp[b, half * TP:(half + 1) * TP] = res.results[core]["out"]
    if _trace:
        kernel.last_results = res
    return outp, x
